# revision 33
# baseline (speedup 1.0000x reference)
"""Multi-head attention (B=4, N=2048, C=1024, H=16) on 8 TRN2 NeuronCores.

Sharding v3: head-parallel. Core c handles batch b = c//2 and head-half
hh = c%2 (8 heads), processing ALL 2048 queries against all 2048 keys.
Q/K/V/proj weights are sliced per head-half on the host, so each
projection is computed exactly once across the pair of cores sharing a
batch (the v2 layout computed K/V twice per batch). Each core emits a
partial output (its heads' contribution through its Wproj rows, plus
bproj/2); the host sums the two partials per batch -- zero on-chip
collectives, one cheap numpy add.

The kernel is a single software-pipelined loop paced by the ScalarE exp
stream and the TensorE matmul stream. Q/K/V/out-proj matmul groups are
deadline-scheduled "filler" work interleaved between the S=K^T Q score
matmuls (row-tiled: both heads of a pair run concurrently in 64x128 PE
tiles), the exp activations, and the P^T V accumulations. Bias additions
ride on the PSUM->SBUF copies. Softmax denominators use the ones-column
trick (row DH of the PV accumulator) and are broadcast via GPSIMD.

Per-core math (all matmul inputs bf16, fp32 PSUM accumulation):
  xT [C, NK] (pre-transposed on host)
  QT = Wq_h.T @ xT (+bq on copy)          [512, NQ]  (feature-major)
  KT = Wk_h.T @ xT (+bk on copy)          [512, NK]
  V  = xT.T @ Wv_h (+bv on copy)          [NK, 512]  (+ones column/head)
  per (head pair p, query chunk qt, key block kg):
    S^T[k, q] = KT_h.T @ QT_h   (contraction dim 64, 2 heads row-tiled)
    P^T = exp(S^T / 8)          (ScalarE, fused scale)
    [out^T_h; rowsum] = [V_h | 1].T @ P^T   (accumulate over 16 k-tiles)
    attnT_h = out^T_h * (1/rowsum) + bv_h
  y_partial = attnT.T @ Wproj_h + bproj/2  [NQ, C]

DMA order is startup-critical-path first: biases, xT tokens 0-511 (all
feature tiles), pair-0 Q/K weight slices, V weights -- so the first
score matmul fires ~12us in instead of ~37us.
"""

import sys

import numpy as np

try:
    import concourse.bacc as bacc
except ImportError:  # pragma: no cover
    sys.path.insert(0, "/opt/trn_rl_repo")
    import concourse.bacc as bacc

import ml_dtypes
import concourse.mybir as mybir
import concourse.tile as tile
from concourse.bass_utils import run_bass_kernel_spmd

bf16 = mybir.dt.bfloat16
f32 = mybir.dt.float32
AF = mybir.ActivationFunctionType

B, N, C = 4, 2048, 1024
H, DH = 16, 64
HL = 8             # heads per core
CL = HL * DH       # feature cols per core (512)
NQ = 2048          # queries per core
NK = 2048          # keys per core
KT = C // 128      # 8 contraction tiles for qkv proj
PT = CL // 128     # 4 contraction tiles for out proj
TT = NK // 128     # 16 key-token tiles
FQ = NQ // 512     # 4 query 512-chunks
VW = DH + 1        # V columns per head incl. ones column
NP = HL // 2       # 4 head pairs
CYC = NP * 8       # cycles per qt pass (32)
LAG = 2            # PV lag behind exp, in pair-cycles

_CACHED = {}


def _build():
    nc = bacc.Bacc()
    xT_d = nc.declare_dram_parameter("xT", [C, NK], bf16, isOutput=False)
    wqkv_d = nc.declare_dram_parameter("wqkv", [C, 3 * CL], bf16, isOutput=False)
    bqkvT_d = nc.declare_dram_parameter("bqkvT", [128, 12], f32, isOutput=False)
    bvb_d = nc.declare_dram_parameter("bvb", [128, CL], f32, isOutput=False)
    bpb_d = nc.declare_dram_parameter("bpb", [128, C], f32, isOutput=False)
    wproj_d = nc.declare_dram_parameter("wproj", [CL, C], bf16, isOutput=False)
    out_d = nc.declare_dram_parameter("out", [NQ, C], f32, isOutput=True)

    with tile.TileContext(nc) as tc:
        from contextlib import ExitStack

        with ExitStack() as ctx:
            perm = ctx.enter_context(tc.tile_pool(name="perm", bufs=1))
            psp = ctx.enter_context(tc.tile_pool(name="psp", bufs=1, space="PSUM"))

            # ---- persistent SBUF ----
            xT = perm.tile([128, KT * NK], bf16)
            xtv = xT[:].rearrange("p (k t) -> p k t", k=KT)
            wv = perm.tile([128, KT * CL], bf16)
            wvv = wv[:].rearrange("p (k f) -> p k f", k=KT)
            wproj = perm.tile([128, PT * C], bf16)
            wpv = wproj[:].rearrange("p (k f) -> p k f", k=PT)
            QT = perm.tile([128, NP * NQ], bf16)
            KTs = perm.tile([128, NP * NK], bf16)
            Vp = perm.tile([128, TT * HL * VW], bf16)
            vpv = Vp[:].rearrange("p (t f) -> p t f", f=VW)
            attnT = perm.tile([128, PT * NQ], bf16)
            bqkvT = perm.tile([128, 12], f32)
            bvb = perm.tile([128, CL], f32)    # V bias broadcast plane
            bpb = perm.tile([128, C], f32)     # proj bias broadcast plane (bproj/2)
            wup = perm.tile([128, 128], bf16)  # junk matmul operand
            wqf = [perm.tile([128, KT * 128], bf16, name=f"wqf{i}") for i in range(NP)]
            wkf = [perm.tile([128, KT * 128], bf16, name=f"wkf{i}") for i in range(NP)]

            nc.vector.memset(wup[:], 0.0)
            nc.vector.memset(vpv[:, :, DH : DH + 1], 1.0)

            # ---- head DMAs, startup-critical-path first ----
            # wsrc/xsrc view [C, *] params as [p, k, col] so one strided DMA
            # fetches a full per-kt slice set (sync-queue issues are ~740ns
            # each, so fewer+bigger wins the startup race).
            wsrc = wqkv_d[:].rearrange("(k p) c -> p k c", k=KT)
            xsrc = xT_d[:].rearrange("(k p) n -> p k n", k=KT)
            psrc = wproj_d[:].rearrange("(k p) c -> p k c", k=PT)
            # Input DMAs fan out across FOUR engine queues (sync, vector,
            # scalar, gpsimd are all idle at startup); issue cost is ~0.7-1us
            # per dma_start per queue, so parallel issue shortens the
            # critical path to the first score matmul by several us.
            nc.sync.dma_start(bqkvT[:], bqkvT_d[:])
            # tokens 0-511 of x (all feature tiles): unblocks Q(ft0,qt0),
            # K(ft0,cg0) and V(tt0-3)
            nc.sync.dma_start(xtv[:, :, 0:512], xsrc[:, :, 0:512])
            nc.scalar.dma_start(
                wqf[0][:].rearrange("p (k f) -> p k f", k=KT), wsrc[:, :, 0:128]
            )
            nc.scalar.dma_start(
                wkf[0][:].rearrange("p (k f) -> p k f", k=KT),
                wsrc[:, :, CL : CL + 128],
            )
            nc.gpsimd.dma_start(wvv[:, :, :], wsrc[:, :, 2 * CL : 3 * CL])
            nc.gpsimd.dma_start(bvb[:], bvb_d[:])
            for t in range(1, 4):
                nc.sync.dma_start(
                    xtv[:, :, t * 512 : (t + 1) * 512], xsrc[:, :, t * 512 : (t + 1) * 512]
                )
            for i in range(1, NP):
                nc.scalar.dma_start(
                    wqf[i][:].rearrange("p (k f) -> p k f", k=KT),
                    wsrc[:, :, i * 128 : (i + 1) * 128],
                )
                nc.gpsimd.dma_start(
                    wkf[i][:].rearrange("p (k f) -> p k f", k=KT),
                    wsrc[:, :, CL + i * 128 : CL + (i + 1) * 128],
                )
            nc.gpsimd.dma_start(wpv[:, :, :], psrc[:, :, :])
            nc.gpsimd.dma_start(bpb[:], bpb_d[:])

            # ---- pipeline pools ----
            pb = ctx.enter_context(tc.tile_pool(name="pb", bufs=1))

            # First gpsimd.partition_broadcast pays a one-time ucode library
            # load (~6us); trigger it on a dummy so it overlaps the input DMAs
            # instead of stalling the first normalize chain.
            gwsrc = pb.tile([1, 512], f32, tag="ri", bufs=2)
            nc.vector.memset(gwsrc[0:1, :], 0.0)
            gwarm = pb.tile([64, 512], f32, tag="bb", bufs=2)
            nc.gpsimd.partition_broadcast(gwarm[:], gwsrc[0:1, :])

            # dummy exp: pulls the ~2.7us ACT table load off the first real
            # exp's critical path (overlaps the input DMAs instead)
            ewarm = pb.tile([1, 16], f32, tag="rc", bufs=2)
            nc.scalar.activation(ewarm[0:1, :], gwsrc[0:1, 0:16], AF.Exp, scale=0.125)

            # warm the PE clock gate while input DMAs land
            jk = psp.tile([128, 512], f32, tag="kp", bufs=2, name="junk")
            for _ in range(64):
                nc.tensor.matmul(jk[:, 0:128], lhsT=wup[:], rhs=wup[:], start=True, stop=True)

            # ---------------- filler machinery ----------------
            # Each filler is a closure emitting ONE engine op (mostly 1 matmul).
            fillers = []   # (deadline, seq, closure), emitted sorted

            def q_group(ft, qt):
                """Q projection for head-pair ft, query chunk qt (512 queries)."""
                box = {}
                wq = wqf[ft][:].rearrange("p (k f) -> p k f", k=KT)
                for k in range(KT):
                    def mm(k=k, box=box, wq=wq, qt=qt, ft=ft):
                        if "ps" not in box:
                            box["ps"] = psp.tile([128, 512], f32, tag="kp", bufs=2,
                                                 name=f"qp{ft}_{qt}")
                        nc.tensor.matmul(
                            box["ps"][:],
                            lhsT=wq[:, k, :],
                            rhs=xtv[:, k, qt * 512 : (qt + 1) * 512],
                            start=(k == 0), stop=(k == KT - 1),
                        )
                    yield mm

                def cp(qt=qt, box=box, ft=ft):
                    nc.vector.tensor_scalar_add(
                        QT[:, ft * NQ + qt * 512 : ft * NQ + qt * 512 + 512],
                        box["ps"][:],
                        bqkvT[:, ft : ft + 1],
                    )
                cp.is_dve = True
                yield cp

            def k_group(ft, cg):
                """K projection for head-pair ft, token chunk cg (512 tokens)."""
                box = {}
                wk = wkf[ft][:].rearrange("p (k f) -> p k f", k=KT)
                for k in range(KT):
                    def mm(k=k, box=box, wk=wk, cg=cg, ft=ft):
                        if "ps" not in box:
                            box["ps"] = psp.tile([128, 512], f32, tag="kp", bufs=2,
                                                 name=f"kp{ft}_{cg}")
                        nc.tensor.matmul(
                            box["ps"][:],
                            lhsT=wk[:, k, :],
                            rhs=xtv[:, k, cg * 512 : (cg + 1) * 512],
                            start=(k == 0), stop=(k == KT - 1),
                        )
                    yield mm

                def cp(box=box, ft=ft, cg=cg):
                    nc.vector.tensor_scalar_add(
                        KTs[:, ft * NK + cg * 512 : ft * NK + cg * 512 + 512],
                        box["ps"][:],
                        bqkvT[:, 4 + ft : 5 + ft],
                    )
                cp.is_dve = True
                yield cp

            def v_group(tt):
                """V projection for token tile tt, all 8 local heads."""
                box = {}
                for k in range(KT):
                    def mm(k=k, box=box, tt=tt):
                        if "ps" not in box:
                            box["ps"] = psp.tile([128, 512], f32, tag="kp", bufs=2,
                                                 name=f"vp{tt}")
                        nc.tensor.matmul(
                            box["ps"][:],
                            lhsT=xtv[:, k, tt * 128 : (tt + 1) * 128],
                            rhs=wvv[:, k, :],
                            start=(k == 0), stop=(k == KT - 1),
                        )
                    yield mm

                def cp(box=box, tt=tt):
                    ob = vpv[:, tt * HL : tt * HL + HL, 0:DH]
                    ib = box["ps"][:].rearrange("p (h d) -> p h d", h=HL)
                    bb_ = bvb[:].rearrange("p (h d) -> p h d", h=HL)
                    nc.vector.tensor_add(ob, ib, bb_)
                cp.is_dve = True
                yield cp

            def proj_group(mt_abs, ks=None, partial_out=None, partial_in=None):
                """Output projection for query block mt_abs (128 rows).

                ks selects attnT feature tiles to contract (default all).
                With partial_out, the result (+bias) is parked in SBUF instead
                of DMA'd; with partial_in, the parked partial is added on the
                way out.
                """
                ks = list(range(PT)) if ks is None else list(ks)
                ybox = {}
                for on in range(2):
                    box = {}
                    for k in ks:
                        def mm(k=k, on=on, box=box, mt_abs=mt_abs, ks=ks):
                            if "ps" not in box:
                                box["ps"] = psp.tile([128, 512], f32, tag="kp", bufs=2,
                                                     name=f"pj{mt_abs}_{on}")
                            nc.tensor.matmul(
                                box["ps"][:],
                                lhsT=attnT[:, k * NQ + mt_abs * 128 : k * NQ + (mt_abs + 1) * 128],
                                rhs=wpv[:, k, on * 512 : (on + 1) * 512],
                                start=(k == ks[0]), stop=(k == ks[-1]),
                            )
                        yield mm

                    def cp(box=box, on=on, mt_abs=mt_abs, ybox=ybox):
                        if on == 0:
                            ybox["yt"] = pb.tile([128, 1024], f32, tag="y", bufs=2,
                                                 name=f"y{mt_abs}")
                        yt = ybox["yt"]
                        nc.vector.tensor_add(
                            yt[:, on * 512 : (on + 1) * 512],
                            box["ps"][:], bpb[:, on * 512 : (on + 1) * 512])
                        if on == 1:
                            # both halves ready: one DMA per query tile (sync
                            # queue issues are ~1us each, so fewer is faster)
                            nc.sync.dma_start(
                                out_d[mt_abs * 128 : (mt_abs + 1) * 128, :],
                                yt[:],
                            )
                    cp.is_dve = True
                    yield cp

            seq = [0]
            pending = []   # (release, deadline, seq, op)

            def add_group(deadline, gen, release=0):
                for op in gen:
                    pending.append((release, deadline, seq[0], op))
                    seq[0] += 1

            # Q: pair ft's qt chunk needed by cycle qt*CYC + ft*8.
            for ft in range(NP):
                for qt in range(FQ):
                    add_group(qt * CYC + ft * 8 - 1, q_group(ft, qt))
            # K: pair ft chunk cg needed by cycle ft*8 + cg*2 (first qt pass).
            for ft in range(NP):
                for cg in range(NK // 512):
                    add_group(ft * 8 + cg * 2 - 1, k_group(ft, cg))
            # V: tile tt consumed from cycle tt//2 + LAG.
            for tt in range(TT):
                add_group(tt // 2 + LAG - 1, v_group(tt))

            def sort_pending():
                pending.sort(key=lambda t: (t[0], t[2]))
                pending.reverse()  # pop() yields lowest release first

            sort_pending()

            def emit_fillers(c, n_base=6):
                while pending and pending[-1][0] <= c:
                    _, dl, sq, op = pending.pop()
                    fillers.append((dl, sq, op))
                fillers.sort(key=lambda t: (t[0], t[1]))
                fillers.reverse()
                # Deadline-due ops are emitted UNCONDITIONALLY: a consumer
                # emitted before its producer copy would get no RAW dependency
                # at all and race it on hardware.
                n = 0
                while fillers and (fillers[-1][0] <= c or n < n_base):
                    _, _, op = fillers.pop()
                    op()
                    n += 1

            # ---------------- main loop ----------------
            pv_q = []          # pending (p, qt, kg, ptA, ptB)
            pots = {}          # head -> ot psum tile

            def emit_pv():
                p2, qt2, kg2, ptJ0, ptJ1 = pv_q.pop(0)
                for hb in range(2):
                    h = 2 * p2 + hb
                    if kg2 == 0:
                        pots[h] = psp.tile([VW, 512], f32, tag="ot", bufs=2, name=f"ot{h}_{qt2}")
                    po = pots[h]
                    for j, ptJ in ((0, ptJ0), (1, ptJ1)):
                        kt = kg2 * 2 + j
                        nc.tensor.matmul(
                            po[:],
                            lhsT=vpv[:, kt * HL + h, :],
                            rhs=ptJ[:, hb * 512 : hb * 512 + 512],
                            start=(kt == 0), stop=(kt == TT - 1),
                        )
                if kg2 == TT // 2 - 1:
                    for h in (2 * p2, 2 * p2 + 1):
                        po = pots.pop(h)
                        ft, bp = h // 2, (h % 2) * 64
                        # Spill po to SBUF immediately so the PSUM slot frees
                        # for the next pair's PV accumulation.
                        pos_t = pb.tile([64, 512], bf16, tag="pos", bufs=3)
                        nc.vector.tensor_copy(pos_t[:], po[0:DH, :])
                        # softmax denominator: 1/rowsum (ones-column of po).
                        # reciprocal_approx_fast misreads PSUM sources -- stage
                        # the row to SBUF first.
                        rc = pb.tile([1, 512], f32, tag="rc", bufs=2)
                        nc.vector.tensor_copy(rc[0:1, :], po[DH : DH + 1, :])
                        ri = pb.tile([1, 512], f32, tag="ri", bufs=2)
                        nc.vector.reciprocal_approx_fast(ri[0:1, :], rc[0:1, :])
                        bb_t = pb.tile([64, 512], f32, tag="bb", bufs=2)
                        nc.gpsimd.partition_broadcast(bb_t[:], ri[0:1, :])
                        # V carries its bias; with softmax rows summing to 1
                        # the normalize is a plain multiply.
                        nc.vector.tensor_mul(
                            attnT[bp : bp + 64,
                                  ft * NQ + qt2 * 512 : ft * NQ + qt2 * 512 + 512],
                            pos_t[:],
                            bb_t[:],
                        )

            c = 0
            for qt in range(FQ):
                for p in range(NP):
                    for kg in range(TT // 2):
                        emit_fillers(c, 6)
                        # scores packed per key-tile: one [128, 1024] psum per
                        # kt holding [head A | head B]. Both heads' matmuls
                        # share identical deps (same psum WAR, same exp), so
                        # the scheduler keeps them adjacent and the 64x128 row
                        # tiles (A at PE rows 0-63, B at 64-127) overlap.
                        pts = []
                        for j in range(2):
                            kt = kg * 2 + j
                            psJ = psp.tile([128, 1024], f32, tag="sc", bufs=2, name=f"sc{c}_{j}")
                            for hb, bp in ((0, 0), (1, 64)):
                                nc.tensor.matmul(
                                    psJ[:, hb * 512 : hb * 512 + 512],
                                    lhsT=KTs[bp : bp + 64,
                                             p * NK + kt * 128 : p * NK + (kt + 1) * 128],
                                    rhs=QT[bp : bp + 64,
                                           p * NQ + qt * 512 : p * NQ + qt * 512 + 512],
                                    start=True, stop=True,
                                )
                            ptJ = pb.tile([128, 1024], bf16, tag="pt", bufs=2 * LAG + 2, name=f"pt{c}_{j}")
                            nc.scalar.activation(ptJ[:], psJ[:], AF.Exp, scale=0.125)
                            pts.append(ptJ)
                        pv_q.append((p, qt, kg, pts[0], pts[1]))
                        if len(pv_q) > LAG:
                            emit_pv()
                        c += 1
                # qt's projection becomes legal once all its normalizes are
                # emitted; schedule it as filler work across the next qt pass.
                if qt < FQ - 1:
                    # staggered releases: the normalize chain the projection
                    # reads (DVE copy -> recip -> gpsimd broadcast -> DVE mul)
                    # has multi-cycle latency after emission; emitting proj
                    # matmuls too early head-of-line blocks the in-order PE
                    # queue on the attnT RAW dependency.
                    for mtl in range(4):
                        add_group(qt * CYC + CYC + 10 + mtl * 5,
                                  proj_group(qt * 4 + mtl),
                                  release=qt * CYC + CYC + LAG + 3 + mtl * 4)
                    sort_pending()
            # drain PV pipeline
            while pv_q:
                emit_pv()
            # keep the PE busy through the last normalize chains' drain
            # (~6us DVE+GPSIMD serial latency): an idle gap >3.4us would
            # re-throttle the HAM clock gate and the tail projections would
            # run at half clock. Fresh pool tile -- writing through the old
            # jk handle would alias a recycled psum slot.
            jk2 = psp.tile([128, 512], f32, tag="kp", bufs=2, name="junk2")
            for _ in range(130):
                nc.tensor.matmul(jk2[:, 0:128], lhsT=wup[:], rhs=wup[:], start=True, stop=True)
            # tail: remaining fillers, then the last qt's projection
            while pending:
                _, dl, sq, op = pending.pop()
                fillers.append((dl, sq, op))
            fillers.sort(key=lambda t: (t[0], t[1]))
            fillers.reverse()
            while fillers:
                _, _, op = fillers.pop()
                op()
            for mt_abs in range(4 * (FQ - 1), 4 * FQ):
                for op in proj_group(mt_abs):
                    op()

    nc.finalize()
    return nc


def _get_nc():
    if "nc" not in _CACHED:
        _CACHED["nc"] = _build()
    return _CACHED["nc"]


def kernel(x, key_padding_mask, Wqkv, bqkv, Wproj, bproj):
    x = np.asarray(x, dtype=np.float32)
    Wqkv = np.asarray(Wqkv, dtype=np.float32)
    bqkv = np.asarray(bqkv, dtype=np.float32)
    Wproj = np.asarray(Wproj, dtype=np.float32)
    bproj = np.asarray(bproj, dtype=np.float32)

    bpb = np.ascontiguousarray(
        np.broadcast_to((bproj * 0.5).reshape(1, C), (128, C))
    ).astype(np.float32)

    xTb = [np.ascontiguousarray(x[b].T).astype(ml_dtypes.bfloat16) for b in range(B)]
    halves = []
    for hh in range(2):
        sl = slice(hh * CL, (hh + 1) * CL)
        wq, wk, wvs = Wqkv[:, sl], Wqkv[:, C + hh * CL : C + (hh + 1) * CL], \
            Wqkv[:, 2 * C + hh * CL : 2 * C + (hh + 1) * CL]
        wqkv_h = np.ascontiguousarray(
            np.concatenate([wq, wk, wvs], axis=1)
        ).astype(ml_dtypes.bfloat16)
        bq = bqkv[hh * CL : (hh + 1) * CL].reshape(NP, 128)
        bk = bqkv[C + hh * CL : C + (hh + 1) * CL].reshape(NP, 128)
        bv = bqkv[2 * C + hh * CL : 2 * C + (hh + 1) * CL]
        bqkvT_h = np.ascontiguousarray(
            np.concatenate([bq, bk, bv.reshape(NP, 128)], axis=0).T
        ).astype(np.float32)
        bvb_h = np.ascontiguousarray(
            np.broadcast_to(bv.reshape(1, CL), (128, CL))
        ).astype(np.float32)
        wproj_h = np.ascontiguousarray(Wproj[hh * CL : (hh + 1) * CL, :]).astype(
            ml_dtypes.bfloat16
        )
        halves.append((wqkv_h, bqkvT_h, bvb_h, wproj_h))

    in_maps = []
    for cc in range(8):
        b, hh = cc // 2, cc % 2
        wqkv_h, bqkvT_h, bvb_h, wproj_h = halves[hh]
        in_maps.append(
            {
                "xT": xTb[b],
                "wqkv": wqkv_h,
                "bqkvT": bqkvT_h,
                "bvb": bvb_h,
                "bpb": bpb,
                "wproj": wproj_h,
            }
        )

    _CACHED["in_maps"] = in_maps
    nc = _get_nc()
    res = run_bass_kernel_spmd(nc, in_maps, core_ids=list(range(8)), trace=False)
    _CACHED["res"] = res

    out = np.empty((B, N, C), dtype=np.float32)
    for b in range(B):
        out[b] = res.results[2 * b]["out"] + res.results[2 * b + 1]["out"]
    return out


# revision 34
# speedup vs baseline: 1.0247x; 1.0247x over previous
"""Multi-head attention (B=4, N=2048, C=1024, H=16) on 8 TRN2 NeuronCores.

Sharding v3: head-parallel. Core c handles batch b = c//2 and head-half
hh = c%2 (8 heads), processing ALL 2048 queries against all 2048 keys.
Q/K/V/proj weights are sliced per head-half on the host, so each
projection is computed exactly once across the pair of cores sharing a
batch (the v2 layout computed K/V twice per batch). Each core emits a
partial output (its heads' contribution through its Wproj rows, plus
bproj/2); the host sums the two partials per batch -- zero on-chip
collectives, one cheap numpy add.

The kernel is a single software-pipelined loop paced by the ScalarE exp
stream and the TensorE matmul stream. Q/K/V/out-proj matmul groups are
deadline-scheduled "filler" work interleaved between the S=K^T Q score
matmuls (row-tiled: both heads of a pair run concurrently in 64x128 PE
tiles), the exp activations, and the P^T V accumulations. Bias additions
ride on the PSUM->SBUF copies. Softmax denominators use the ones-column
trick (row DH of the PV accumulator) and are broadcast via GPSIMD.

Per-core math (all matmul inputs bf16, fp32 PSUM accumulation):
  xT [C, NK] (pre-transposed on host)
  QT = Wq_h.T @ xT (+bq on copy)          [512, NQ]  (feature-major)
  KT = Wk_h.T @ xT (+bk on copy)          [512, NK]
  V  = xT.T @ Wv_h (+bv on copy)          [NK, 512]  (+ones column/head)
  per (head pair p, query chunk qt, key block kg):
    S^T[k, q] = KT_h.T @ QT_h   (contraction dim 64, 2 heads row-tiled)
    P^T = exp(S^T / 8)          (ScalarE, fused scale)
    [out^T_h; rowsum] = [V_h | 1].T @ P^T   (accumulate over 16 k-tiles)
    attnT_h = out^T_h * (1/rowsum) + bv_h
  y_partial = attnT.T @ Wproj_h + bproj/2  [NQ, C]

DMA order is startup-critical-path first: biases, xT tokens 0-511 (all
feature tiles), pair-0 Q/K weight slices, V weights -- so the first
score matmul fires ~12us in instead of ~37us.
"""

import sys

import numpy as np

try:
    import concourse.bacc as bacc
except ImportError:  # pragma: no cover
    sys.path.insert(0, "/opt/trn_rl_repo")
    import concourse.bacc as bacc

import ml_dtypes
import concourse.mybir as mybir
import concourse.tile as tile
from concourse.bass_utils import run_bass_kernel_spmd

bf16 = mybir.dt.bfloat16
f32 = mybir.dt.float32
AF = mybir.ActivationFunctionType

B, N, C = 4, 2048, 1024
H, DH = 16, 64
HL = 8             # heads per core
CL = HL * DH       # feature cols per core (512)
NQ = 2048          # queries per core
NK = 2048          # keys per core
KT = C // 128      # 8 contraction tiles for qkv proj
PT = CL // 128     # 4 contraction tiles for out proj
TT = NK // 128     # 16 key-token tiles
FQ = NQ // 512     # 4 query 512-chunks
VW = DH + 1        # V columns per head incl. ones column
NP = HL // 2       # 4 head pairs
CYC = NP * 8       # cycles per qt pass (32)
LAG = 2            # PV lag behind exp, in pair-cycles

_CACHED = {}


def _build():
    nc = bacc.Bacc()
    xT_d = nc.declare_dram_parameter("xT", [C, NK], bf16, isOutput=False)
    wqkv_d = nc.declare_dram_parameter("wqkv", [C, 3 * CL], bf16, isOutput=False)
    bqkvT_d = nc.declare_dram_parameter("bqkvT", [128, 12], f32, isOutput=False)
    bvb_d = nc.declare_dram_parameter("bvb", [128, CL], f32, isOutput=False)
    bpb_d = nc.declare_dram_parameter("bpb", [128, C], f32, isOutput=False)
    wproj_d = nc.declare_dram_parameter("wproj", [CL, C], bf16, isOutput=False)
    out_d = nc.declare_dram_parameter("out", [NQ, C], f32, isOutput=True)

    with tile.TileContext(nc) as tc:
        from contextlib import ExitStack

        with ExitStack() as ctx:
            perm = ctx.enter_context(tc.tile_pool(name="perm", bufs=1))
            psp = ctx.enter_context(tc.tile_pool(name="psp", bufs=1, space="PSUM"))

            # ---- persistent SBUF ----
            xT = perm.tile([128, KT * NK], bf16)
            xtv = xT[:].rearrange("p (k t) -> p k t", k=KT)
            wv = perm.tile([128, KT * CL], bf16)
            wvv = wv[:].rearrange("p (k f) -> p k f", k=KT)
            wproj = perm.tile([128, PT * C], bf16)
            wpv = wproj[:].rearrange("p (k f) -> p k f", k=PT)
            QT = perm.tile([128, NP * NQ], bf16)
            KTs = perm.tile([128, NP * NK], bf16)
            Vp = perm.tile([128, TT * HL * VW], bf16)
            vpv = Vp[:].rearrange("p (t f) -> p t f", f=VW)
            attnT = perm.tile([128, PT * NQ], bf16)
            bqkvT = perm.tile([128, 12], f32)
            bvb = perm.tile([128, CL], f32)    # V bias broadcast plane
            bpb = perm.tile([128, C], f32)     # proj bias broadcast plane (bproj/2)
            wup = perm.tile([128, 128], bf16)  # junk matmul operand
            wqf = [perm.tile([128, KT * 128], bf16, name=f"wqf{i}") for i in range(NP)]
            wkf = [perm.tile([128, KT * 128], bf16, name=f"wkf{i}") for i in range(NP)]

            nc.vector.memset(wup[:], 0.0)
            nc.vector.memset(vpv[:, :, DH : DH + 1], 1.0)

            # ---- head DMAs, startup-critical-path first ----
            # wsrc/xsrc view [C, *] params as [p, k, col] so one strided DMA
            # fetches a full per-kt slice set (sync-queue issues are ~740ns
            # each, so fewer+bigger wins the startup race).
            wsrc = wqkv_d[:].rearrange("(k p) c -> p k c", k=KT)
            xsrc = xT_d[:].rearrange("(k p) n -> p k n", k=KT)
            psrc = wproj_d[:].rearrange("(k p) c -> p k c", k=PT)
            nc.sync.dma_start(bqkvT[:], bqkvT_d[:])
            # tokens 0-511 of x (all feature tiles): unblocks Q(ft0,qt0),
            # K(ft0,cg0) and V(tt0-3)
            nc.sync.dma_start(xtv[:, :, 0:512], xsrc[:, :, 0:512])
            nc.sync.dma_start(
                wqf[0][:].rearrange("p (k f) -> p k f", k=KT), wsrc[:, :, 0:128]
            )
            nc.sync.dma_start(
                wkf[0][:].rearrange("p (k f) -> p k f", k=KT),
                wsrc[:, :, CL : CL + 128],
            )
            nc.sync.dma_start(wvv[:, :, :], wsrc[:, :, 2 * CL : 3 * CL])
            nc.sync.dma_start(bvb[:], bvb_d[:])
            for t in range(1, 4):
                nc.sync.dma_start(
                    xtv[:, :, t * 512 : (t + 1) * 512], xsrc[:, :, t * 512 : (t + 1) * 512]
                )
            for i in range(1, NP):
                nc.sync.dma_start(
                    wqf[i][:].rearrange("p (k f) -> p k f", k=KT),
                    wsrc[:, :, i * 128 : (i + 1) * 128],
                )
                nc.sync.dma_start(
                    wkf[i][:].rearrange("p (k f) -> p k f", k=KT),
                    wsrc[:, :, CL + i * 128 : CL + (i + 1) * 128],
                )
            nc.sync.dma_start(wpv[:, :, :], psrc[:, :, :])
            nc.sync.dma_start(bpb[:], bpb_d[:])

            # ---- pipeline pools ----
            pb = ctx.enter_context(tc.tile_pool(name="pb", bufs=1))

            # First gpsimd.partition_broadcast pays a one-time ucode library
            # load (~6us); trigger it on a dummy so it overlaps the input DMAs
            # instead of stalling the first normalize chain.
            gwsrc = pb.tile([1, 512], f32, tag="ri", bufs=2)
            nc.vector.memset(gwsrc[0:1, :], 0.0)
            gwarm = pb.tile([64, 512], f32, tag="bb", bufs=2)
            nc.gpsimd.partition_broadcast(gwarm[:], gwsrc[0:1, :])

            # dummy exp: pulls the ~2.7us ACT table load off the first real
            # exp's critical path (overlaps the input DMAs instead)
            ewarm = pb.tile([1, 16], f32, tag="rc", bufs=2)
            nc.scalar.activation(ewarm[0:1, :], gwsrc[0:1, 0:16], AF.Exp, scale=0.125)

            # warm the PE clock gate while input DMAs land
            jk = psp.tile([128, 512], f32, tag="kp", bufs=2, name="junk")
            for _ in range(64):
                nc.tensor.matmul(jk[:, 0:128], lhsT=wup[:], rhs=wup[:], start=True, stop=True)

            # ---------------- filler machinery ----------------
            # Each filler is a closure emitting ONE engine op (mostly 1 matmul).
            fillers = []   # (deadline, seq, closure), emitted sorted

            def q_group(ft, qt):
                """Q projection for head-pair ft, query chunk qt (512 queries)."""
                box = {}
                wq = wqf[ft][:].rearrange("p (k f) -> p k f", k=KT)
                for k in range(KT):
                    def mm(k=k, box=box, wq=wq, qt=qt, ft=ft):
                        if "ps" not in box:
                            box["ps"] = psp.tile([128, 512], f32, tag="kp", bufs=2,
                                                 name=f"qp{ft}_{qt}")
                        nc.tensor.matmul(
                            box["ps"][:],
                            lhsT=wq[:, k, :],
                            rhs=xtv[:, k, qt * 512 : (qt + 1) * 512],
                            start=(k == 0), stop=(k == KT - 1),
                        )
                    yield mm

                def cp(qt=qt, box=box, ft=ft):
                    nc.vector.tensor_scalar_add(
                        QT[:, ft * NQ + qt * 512 : ft * NQ + qt * 512 + 512],
                        box["ps"][:],
                        bqkvT[:, ft : ft + 1],
                    )
                cp.is_dve = True
                yield cp

            def k_group(ft, cg):
                """K projection for head-pair ft, token chunk cg (512 tokens)."""
                box = {}
                wk = wkf[ft][:].rearrange("p (k f) -> p k f", k=KT)
                for k in range(KT):
                    def mm(k=k, box=box, wk=wk, cg=cg, ft=ft):
                        if "ps" not in box:
                            box["ps"] = psp.tile([128, 512], f32, tag="kp", bufs=2,
                                                 name=f"kp{ft}_{cg}")
                        nc.tensor.matmul(
                            box["ps"][:],
                            lhsT=wk[:, k, :],
                            rhs=xtv[:, k, cg * 512 : (cg + 1) * 512],
                            start=(k == 0), stop=(k == KT - 1),
                        )
                    yield mm

                def cp(box=box, ft=ft, cg=cg):
                    nc.vector.tensor_scalar_add(
                        KTs[:, ft * NK + cg * 512 : ft * NK + cg * 512 + 512],
                        box["ps"][:],
                        bqkvT[:, 4 + ft : 5 + ft],
                    )
                cp.is_dve = True
                yield cp

            def v_group(tt):
                """V projection for token tile tt, all 8 local heads."""
                box = {}
                for k in range(KT):
                    def mm(k=k, box=box, tt=tt):
                        if "ps" not in box:
                            box["ps"] = psp.tile([128, 512], f32, tag="kp", bufs=2,
                                                 name=f"vp{tt}")
                        nc.tensor.matmul(
                            box["ps"][:],
                            lhsT=xtv[:, k, tt * 128 : (tt + 1) * 128],
                            rhs=wvv[:, k, :],
                            start=(k == 0), stop=(k == KT - 1),
                        )
                    yield mm

                def cp(box=box, tt=tt):
                    ob = vpv[:, tt * HL : tt * HL + HL, 0:DH]
                    ib = box["ps"][:].rearrange("p (h d) -> p h d", h=HL)
                    bb_ = bvb[:].rearrange("p (h d) -> p h d", h=HL)
                    nc.vector.tensor_add(ob, ib, bb_)
                cp.is_dve = True
                yield cp

            def proj_group(mt_abs, ks=None, partial_out=None, partial_in=None):
                """Output projection for query block mt_abs (128 rows).

                ks selects attnT feature tiles to contract (default all).
                With partial_out, the result (+bias) is parked in SBUF instead
                of DMA'd; with partial_in, the parked partial is added on the
                way out.
                """
                ks = list(range(PT)) if ks is None else list(ks)
                ybox = {}
                for on in range(2):
                    box = {}
                    for k in ks:
                        def mm(k=k, on=on, box=box, mt_abs=mt_abs, ks=ks):
                            if "ps" not in box:
                                box["ps"] = psp.tile([128, 512], f32, tag="kp", bufs=2,
                                                     name=f"pj{mt_abs}_{on}")
                            nc.tensor.matmul(
                                box["ps"][:],
                                lhsT=attnT[:, k * NQ + mt_abs * 128 : k * NQ + (mt_abs + 1) * 128],
                                rhs=wpv[:, k, on * 512 : (on + 1) * 512],
                                start=(k == ks[0]), stop=(k == ks[-1]),
                            )
                        yield mm

                    def cp(box=box, on=on, mt_abs=mt_abs, ybox=ybox):
                        if on == 0:
                            ybox["yt"] = pb.tile([128, 1024], f32, tag="y", bufs=2,
                                                 name=f"y{mt_abs}")
                        yt = ybox["yt"]
                        nc.vector.tensor_add(
                            yt[:, on * 512 : (on + 1) * 512],
                            box["ps"][:], bpb[:, on * 512 : (on + 1) * 512])
                        if on == 1:
                            # both halves ready: one DMA per query tile (sync
                            # queue issues are ~1us each, so fewer is faster)
                            nc.sync.dma_start(
                                out_d[mt_abs * 128 : (mt_abs + 1) * 128, :],
                                yt[:],
                            )
                    cp.is_dve = True
                    yield cp

            seq = [0]
            pending = []   # (release, deadline, seq, op)

            def add_group(deadline, gen, release=0):
                for op in gen:
                    pending.append((release, deadline, seq[0], op))
                    seq[0] += 1

            # Q: pair ft's qt chunk needed by cycle qt*CYC + ft*8.
            for ft in range(NP):
                for qt in range(FQ):
                    add_group(qt * CYC + ft * 8 - 1, q_group(ft, qt))
            # K: pair ft chunk cg needed by cycle ft*8 + cg*2 (first qt pass).
            for ft in range(NP):
                for cg in range(NK // 512):
                    add_group(ft * 8 + cg * 2 - 1, k_group(ft, cg))
            # V: tile tt consumed from cycle tt//2 + LAG.
            for tt in range(TT):
                add_group(tt // 2 + LAG - 1, v_group(tt))

            def sort_pending():
                pending.sort(key=lambda t: (t[0], t[2]))
                pending.reverse()  # pop() yields lowest release first

            sort_pending()

            def emit_fillers(c, n_base=6):
                while pending and pending[-1][0] <= c:
                    _, dl, sq, op = pending.pop()
                    fillers.append((dl, sq, op))
                fillers.sort(key=lambda t: (t[0], t[1]))
                fillers.reverse()
                # Deadline-due ops are emitted UNCONDITIONALLY: a consumer
                # emitted before its producer copy would get no RAW dependency
                # at all and race it on hardware.
                n = 0
                while fillers and (fillers[-1][0] <= c or n < n_base):
                    _, _, op = fillers.pop()
                    op()
                    n += 1

            # ---------------- main loop ----------------
            pv_q = []          # pending (p, qt, kg, ptA, ptB)
            pots = {}          # head -> ot psum tile

            def emit_pv():
                p2, qt2, kg2, ptJ0, ptJ1 = pv_q.pop(0)
                for hb in range(2):
                    h = 2 * p2 + hb
                    if kg2 == 0:
                        pots[h] = psp.tile([VW, 512], f32, tag="ot", bufs=2, name=f"ot{h}_{qt2}")
                    po = pots[h]
                    for j, ptJ in ((0, ptJ0), (1, ptJ1)):
                        kt = kg2 * 2 + j
                        nc.tensor.matmul(
                            po[:],
                            lhsT=vpv[:, kt * HL + h, :],
                            rhs=ptJ[:, hb * 512 : hb * 512 + 512],
                            start=(kt == 0), stop=(kt == TT - 1),
                        )
                if kg2 == TT // 2 - 1:
                    for h in (2 * p2, 2 * p2 + 1):
                        po = pots.pop(h)
                        ft, bp = h // 2, (h % 2) * 64
                        # Spill po to SBUF immediately so the PSUM slot frees
                        # for the next pair's PV accumulation.
                        pos_t = pb.tile([64, 512], bf16, tag="pos", bufs=3)
                        nc.vector.tensor_copy(pos_t[:], po[0:DH, :])
                        # softmax denominator: 1/rowsum (ones-column of po).
                        # reciprocal_approx_fast misreads PSUM sources -- stage
                        # the row to SBUF first.
                        rc = pb.tile([1, 512], f32, tag="rc", bufs=2)
                        nc.vector.tensor_copy(rc[0:1, :], po[DH : DH + 1, :])
                        ri = pb.tile([1, 512], f32, tag="ri", bufs=2)
                        nc.vector.reciprocal_approx_fast(ri[0:1, :], rc[0:1, :])
                        bb_t = pb.tile([64, 512], f32, tag="bb", bufs=2)
                        nc.gpsimd.partition_broadcast(bb_t[:], ri[0:1, :])
                        # V carries its bias; with softmax rows summing to 1
                        # the normalize is a plain multiply.
                        nc.vector.tensor_mul(
                            attnT[bp : bp + 64,
                                  ft * NQ + qt2 * 512 : ft * NQ + qt2 * 512 + 512],
                            pos_t[:],
                            bb_t[:],
                        )

            c = 0
            for qt in range(FQ):
                for p in range(NP):
                    for kg in range(TT // 2):
                        emit_fillers(c, 6)
                        # scores packed per key-tile: one [128, 1024] psum per
                        # kt holding [head A | head B]. Both heads' matmuls
                        # share identical deps (same psum WAR, same exp), so
                        # the scheduler keeps them adjacent and the 64x128 row
                        # tiles (A at PE rows 0-63, B at 64-127) overlap.
                        pts = []
                        for j in range(2):
                            kt = kg * 2 + j
                            psJ = psp.tile([128, 1024], f32, tag="sc", bufs=2, name=f"sc{c}_{j}")
                            for hb, bp in ((0, 0), (1, 64)):
                                nc.tensor.matmul(
                                    psJ[:, hb * 512 : hb * 512 + 512],
                                    lhsT=KTs[bp : bp + 64,
                                             p * NK + kt * 128 : p * NK + (kt + 1) * 128],
                                    rhs=QT[bp : bp + 64,
                                           p * NQ + qt * 512 : p * NQ + qt * 512 + 512],
                                    start=True, stop=True,
                                )
                            ptJ = pb.tile([128, 1024], bf16, tag="pt", bufs=2 * LAG + 2, name=f"pt{c}_{j}")
                            nc.scalar.activation(ptJ[:], psJ[:], AF.Exp, scale=0.125)
                            pts.append(ptJ)
                        pv_q.append((p, qt, kg, pts[0], pts[1]))
                        if len(pv_q) > LAG:
                            emit_pv()
                        c += 1
                # qt's projection becomes legal once all its normalizes are
                # emitted; schedule it as filler work across the next qt pass.
                if qt < FQ - 1:
                    # staggered releases: the normalize chain the projection
                    # reads (DVE copy -> recip -> gpsimd broadcast -> DVE mul)
                    # has multi-cycle latency after emission; emitting proj
                    # matmuls too early head-of-line blocks the in-order PE
                    # queue on the attnT RAW dependency.
                    for mtl in range(4):
                        add_group(qt * CYC + CYC + 10 + mtl * 5,
                                  proj_group(qt * 4 + mtl),
                                  release=qt * CYC + CYC + LAG + 3 + mtl * 4)
                    sort_pending()
            # drain PV pipeline
            while pv_q:
                emit_pv()
            # keep the PE busy through the last normalize chains' drain
            # (~6us DVE+GPSIMD serial latency): an idle gap >3.4us would
            # re-throttle the HAM clock gate and the tail projections would
            # run at half clock. Fresh pool tile -- writing through the old
            # jk handle would alias a recycled psum slot.
            jk2 = psp.tile([128, 512], f32, tag="kp", bufs=2, name="junk2")
            for _ in range(130):
                nc.tensor.matmul(jk2[:, 0:128], lhsT=wup[:], rhs=wup[:], start=True, stop=True)
            # tail: remaining fillers, then the last qt's projection
            while pending:
                _, dl, sq, op = pending.pop()
                fillers.append((dl, sq, op))
            fillers.sort(key=lambda t: (t[0], t[1]))
            fillers.reverse()
            while fillers:
                _, _, op = fillers.pop()
                op()
            for mt_abs in range(4 * (FQ - 1), 4 * FQ):
                for op in proj_group(mt_abs):
                    op()

    nc.finalize()
    return nc


def _get_nc():
    if "nc" not in _CACHED:
        _CACHED["nc"] = _build()
    return _CACHED["nc"]


def kernel(x, key_padding_mask, Wqkv, bqkv, Wproj, bproj):
    x = np.asarray(x, dtype=np.float32)
    Wqkv = np.asarray(Wqkv, dtype=np.float32)
    bqkv = np.asarray(bqkv, dtype=np.float32)
    Wproj = np.asarray(Wproj, dtype=np.float32)
    bproj = np.asarray(bproj, dtype=np.float32)

    bpb = np.ascontiguousarray(
        np.broadcast_to((bproj * 0.5).reshape(1, C), (128, C))
    ).astype(np.float32)

    xTb = [np.ascontiguousarray(x[b].T).astype(ml_dtypes.bfloat16) for b in range(B)]
    halves = []
    for hh in range(2):
        sl = slice(hh * CL, (hh + 1) * CL)
        wq, wk, wvs = Wqkv[:, sl], Wqkv[:, C + hh * CL : C + (hh + 1) * CL], \
            Wqkv[:, 2 * C + hh * CL : 2 * C + (hh + 1) * CL]
        wqkv_h = np.ascontiguousarray(
            np.concatenate([wq, wk, wvs], axis=1)
        ).astype(ml_dtypes.bfloat16)
        bq = bqkv[hh * CL : (hh + 1) * CL].reshape(NP, 128)
        bk = bqkv[C + hh * CL : C + (hh + 1) * CL].reshape(NP, 128)
        bv = bqkv[2 * C + hh * CL : 2 * C + (hh + 1) * CL]
        bqkvT_h = np.ascontiguousarray(
            np.concatenate([bq, bk, bv.reshape(NP, 128)], axis=0).T
        ).astype(np.float32)
        bvb_h = np.ascontiguousarray(
            np.broadcast_to(bv.reshape(1, CL), (128, CL))
        ).astype(np.float32)
        wproj_h = np.ascontiguousarray(Wproj[hh * CL : (hh + 1) * CL, :]).astype(
            ml_dtypes.bfloat16
        )
        halves.append((wqkv_h, bqkvT_h, bvb_h, wproj_h))

    in_maps = []
    for cc in range(8):
        b, hh = cc // 2, cc % 2
        wqkv_h, bqkvT_h, bvb_h, wproj_h = halves[hh]
        in_maps.append(
            {
                "xT": xTb[b],
                "wqkv": wqkv_h,
                "bqkvT": bqkvT_h,
                "bvb": bvb_h,
                "bpb": bpb,
                "wproj": wproj_h,
            }
        )

    _CACHED["in_maps"] = in_maps
    nc = _get_nc()
    res = run_bass_kernel_spmd(nc, in_maps, core_ids=list(range(8)), trace=False)
    _CACHED["res"] = res

    out = np.empty((B, N, C), dtype=np.float32)
    for b in range(B):
        out[b] = res.results[2 * b]["out"] + res.results[2 * b + 1]["out"]
    return out


# revision 35
# speedup vs baseline: 1.0283x; 1.0036x over previous
"""Multi-head attention (B=4, N=2048, C=1024, H=16) on 8 TRN2 NeuronCores.

Sharding v3: head-parallel. Core c handles batch b = c//2 and head-half
hh = c%2 (8 heads), processing ALL 2048 queries against all 2048 keys.
Q/K/V/proj weights are sliced per head-half on the host, so each
projection is computed exactly once across the pair of cores sharing a
batch (the v2 layout computed K/V twice per batch). Each core emits a
partial output (its heads' contribution through its Wproj rows, plus
bproj/2); the host sums the two partials per batch -- zero on-chip
collectives, one cheap numpy add.

The kernel is a single software-pipelined loop paced by the ScalarE exp
stream and the TensorE matmul stream. Q/K/V/out-proj matmul groups are
deadline-scheduled "filler" work interleaved between the S=K^T Q score
matmuls (row-tiled: both heads of a pair run concurrently in 64x128 PE
tiles), the exp activations, and the P^T V accumulations. Bias additions
ride on the PSUM->SBUF copies. Softmax denominators use the ones-column
trick (row DH of the PV accumulator) and are broadcast via GPSIMD.

Per-core math (all matmul inputs bf16, fp32 PSUM accumulation):
  xT [C, NK] (pre-transposed on host)
  QT = Wq_h.T @ xT (+bq on copy)          [512, NQ]  (feature-major)
  KT = Wk_h.T @ xT (+bk on copy)          [512, NK]
  V  = xT.T @ Wv_h (+bv on copy)          [NK, 512]  (+ones column/head)
  per (head pair p, query chunk qt, key block kg):
    S^T[k, q] = KT_h.T @ QT_h   (contraction dim 64, 2 heads row-tiled)
    P^T = exp(S^T / 8)          (ScalarE, fused scale)
    [out^T_h; rowsum] = [V_h | 1].T @ P^T   (accumulate over 16 k-tiles)
    attnT_h = out^T_h * (1/rowsum) + bv_h
  y_partial = attnT.T @ Wproj_h + bproj/2  [NQ, C]

DMA order is startup-critical-path first: biases, xT tokens 0-511 (all
feature tiles), pair-0 Q/K weight slices, V weights -- so the first
score matmul fires ~12us in instead of ~37us.
"""

import sys

import numpy as np

try:
    import concourse.bacc as bacc
except ImportError:  # pragma: no cover
    sys.path.insert(0, "/opt/trn_rl_repo")
    import concourse.bacc as bacc

import ml_dtypes
import concourse.mybir as mybir
import concourse.tile as tile
from concourse.bass_utils import run_bass_kernel_spmd

bf16 = mybir.dt.bfloat16
f32 = mybir.dt.float32
AF = mybir.ActivationFunctionType

B, N, C = 4, 2048, 1024
H, DH = 16, 64
HL = 8             # heads per core
CL = HL * DH       # feature cols per core (512)
NQ = 2048          # queries per core
NK = 2048          # keys per core
KT = C // 128      # 8 contraction tiles for qkv proj
PT = CL // 128     # 4 contraction tiles for out proj
TT = NK // 128     # 16 key-token tiles
FQ = NQ // 512     # 4 query 512-chunks
VW = DH + 1        # V columns per head incl. ones column
NP = HL // 2       # 4 head pairs
CYC = NP * 8       # cycles per qt pass (32)
LAG = 3            # PV lag behind exp, in pair-cycles

_CACHED = {}


def _build():
    nc = bacc.Bacc()
    xT_d = nc.declare_dram_parameter("xT", [C, NK], bf16, isOutput=False)
    wqkv_d = nc.declare_dram_parameter("wqkv", [C, 3 * CL], bf16, isOutput=False)
    bqkvT_d = nc.declare_dram_parameter("bqkvT", [128, 12], f32, isOutput=False)
    bvb_d = nc.declare_dram_parameter("bvb", [128, CL], f32, isOutput=False)
    bpb_d = nc.declare_dram_parameter("bpb", [128, C], f32, isOutput=False)
    wproj_d = nc.declare_dram_parameter("wproj", [CL, C], bf16, isOutput=False)
    out_d = nc.declare_dram_parameter("out", [NQ, C], f32, isOutput=True)

    with tile.TileContext(nc) as tc:
        from contextlib import ExitStack

        with ExitStack() as ctx:
            perm = ctx.enter_context(tc.tile_pool(name="perm", bufs=1))
            psp = ctx.enter_context(tc.tile_pool(name="psp", bufs=1, space="PSUM"))

            # ---- persistent SBUF ----
            xT = perm.tile([128, KT * NK], bf16)
            xtv = xT[:].rearrange("p (k t) -> p k t", k=KT)
            wv = perm.tile([128, KT * CL], bf16)
            wvv = wv[:].rearrange("p (k f) -> p k f", k=KT)
            wproj = perm.tile([128, PT * C], bf16)
            wpv = wproj[:].rearrange("p (k f) -> p k f", k=PT)
            QT = perm.tile([128, NP * NQ], bf16)
            KTs = perm.tile([128, NP * NK], bf16)
            Vp = perm.tile([128, TT * HL * VW], bf16)
            vpv = Vp[:].rearrange("p (t f) -> p t f", f=VW)
            attnT = perm.tile([128, PT * NQ], bf16)
            bqkvT = perm.tile([128, 12], f32)
            bvb = perm.tile([128, CL], f32)    # V bias broadcast plane
            bpb = perm.tile([128, C], f32)     # proj bias broadcast plane (bproj/2)
            wup = perm.tile([128, 128], bf16)  # junk matmul operand
            wqf = [perm.tile([128, KT * 128], bf16, name=f"wqf{i}") for i in range(NP)]
            wkf = [perm.tile([128, KT * 128], bf16, name=f"wkf{i}") for i in range(NP)]

            nc.vector.memset(wup[:], 0.0)
            nc.vector.memset(vpv[:, :, DH : DH + 1], 1.0)

            # ---- head DMAs, startup-critical-path first ----
            # wsrc/xsrc view [C, *] params as [p, k, col] so one strided DMA
            # fetches a full per-kt slice set (sync-queue issues are ~740ns
            # each, so fewer+bigger wins the startup race).
            wsrc = wqkv_d[:].rearrange("(k p) c -> p k c", k=KT)
            xsrc = xT_d[:].rearrange("(k p) n -> p k n", k=KT)
            psrc = wproj_d[:].rearrange("(k p) c -> p k c", k=PT)
            nc.sync.dma_start(bqkvT[:], bqkvT_d[:])
            # tokens 0-511 of x (all feature tiles): unblocks Q(ft0,qt0),
            # K(ft0,cg0) and V(tt0-3)
            nc.sync.dma_start(xtv[:, :, 0:512], xsrc[:, :, 0:512])
            nc.sync.dma_start(
                wqf[0][:].rearrange("p (k f) -> p k f", k=KT), wsrc[:, :, 0:128]
            )
            nc.sync.dma_start(
                wkf[0][:].rearrange("p (k f) -> p k f", k=KT),
                wsrc[:, :, CL : CL + 128],
            )
            nc.sync.dma_start(wvv[:, :, :], wsrc[:, :, 2 * CL : 3 * CL])
            nc.sync.dma_start(bvb[:], bvb_d[:])
            for t in range(1, 4):
                nc.sync.dma_start(
                    xtv[:, :, t * 512 : (t + 1) * 512], xsrc[:, :, t * 512 : (t + 1) * 512]
                )
            for i in range(1, NP):
                nc.sync.dma_start(
                    wqf[i][:].rearrange("p (k f) -> p k f", k=KT),
                    wsrc[:, :, i * 128 : (i + 1) * 128],
                )
                nc.sync.dma_start(
                    wkf[i][:].rearrange("p (k f) -> p k f", k=KT),
                    wsrc[:, :, CL + i * 128 : CL + (i + 1) * 128],
                )
            nc.sync.dma_start(wpv[:, :, :], psrc[:, :, :])
            nc.sync.dma_start(bpb[:], bpb_d[:])

            # ---- pipeline pools ----
            pb = ctx.enter_context(tc.tile_pool(name="pb", bufs=1))

            # First gpsimd.partition_broadcast pays a one-time ucode library
            # load (~6us); trigger it on a dummy so it overlaps the input DMAs
            # instead of stalling the first normalize chain.
            gwsrc = pb.tile([1, 512], f32, tag="ri", bufs=2)
            nc.vector.memset(gwsrc[0:1, :], 0.0)
            gwarm = pb.tile([64, 512], f32, tag="bb", bufs=2)
            nc.gpsimd.partition_broadcast(gwarm[:], gwsrc[0:1, :])

            # dummy exp: pulls the ~2.7us ACT table load off the first real
            # exp's critical path (overlaps the input DMAs instead)
            ewarm = pb.tile([1, 16], f32, tag="rc", bufs=2)
            nc.scalar.activation(ewarm[0:1, :], gwsrc[0:1, 0:16], AF.Exp, scale=0.125)

            # warm the PE clock gate while input DMAs land
            jk = psp.tile([128, 512], f32, tag="kp", bufs=2, name="junk")
            for _ in range(64):
                nc.tensor.matmul(jk[:, 0:128], lhsT=wup[:], rhs=wup[:], start=True, stop=True)

            # ---------------- filler machinery ----------------
            # Each filler is a closure emitting ONE engine op (mostly 1 matmul).
            fillers = []   # (deadline, seq, closure), emitted sorted

            def q_group(ft, qt):
                """Q projection for head-pair ft, query chunk qt (512 queries)."""
                box = {}
                wq = wqf[ft][:].rearrange("p (k f) -> p k f", k=KT)
                for k in range(KT):
                    def mm(k=k, box=box, wq=wq, qt=qt, ft=ft):
                        if "ps" not in box:
                            box["ps"] = psp.tile([128, 512], f32, tag="kp", bufs=2,
                                                 name=f"qp{ft}_{qt}")
                        nc.tensor.matmul(
                            box["ps"][:],
                            lhsT=wq[:, k, :],
                            rhs=xtv[:, k, qt * 512 : (qt + 1) * 512],
                            start=(k == 0), stop=(k == KT - 1),
                        )
                    yield mm

                def cp(qt=qt, box=box, ft=ft):
                    nc.vector.tensor_scalar_add(
                        QT[:, ft * NQ + qt * 512 : ft * NQ + qt * 512 + 512],
                        box["ps"][:],
                        bqkvT[:, ft : ft + 1],
                    )
                cp.is_dve = True
                yield cp

            def k_group(ft, cg):
                """K projection for head-pair ft, token chunk cg (512 tokens)."""
                box = {}
                wk = wkf[ft][:].rearrange("p (k f) -> p k f", k=KT)
                for k in range(KT):
                    def mm(k=k, box=box, wk=wk, cg=cg, ft=ft):
                        if "ps" not in box:
                            box["ps"] = psp.tile([128, 512], f32, tag="kp", bufs=2,
                                                 name=f"kp{ft}_{cg}")
                        nc.tensor.matmul(
                            box["ps"][:],
                            lhsT=wk[:, k, :],
                            rhs=xtv[:, k, cg * 512 : (cg + 1) * 512],
                            start=(k == 0), stop=(k == KT - 1),
                        )
                    yield mm

                def cp(box=box, ft=ft, cg=cg):
                    nc.vector.tensor_scalar_add(
                        KTs[:, ft * NK + cg * 512 : ft * NK + cg * 512 + 512],
                        box["ps"][:],
                        bqkvT[:, 4 + ft : 5 + ft],
                    )
                cp.is_dve = True
                yield cp

            def v_group(tt):
                """V projection for token tile tt, all 8 local heads."""
                box = {}
                for k in range(KT):
                    def mm(k=k, box=box, tt=tt):
                        if "ps" not in box:
                            box["ps"] = psp.tile([128, 512], f32, tag="kp", bufs=2,
                                                 name=f"vp{tt}")
                        nc.tensor.matmul(
                            box["ps"][:],
                            lhsT=xtv[:, k, tt * 128 : (tt + 1) * 128],
                            rhs=wvv[:, k, :],
                            start=(k == 0), stop=(k == KT - 1),
                        )
                    yield mm

                def cp(box=box, tt=tt):
                    ob = vpv[:, tt * HL : tt * HL + HL, 0:DH]
                    ib = box["ps"][:].rearrange("p (h d) -> p h d", h=HL)
                    bb_ = bvb[:].rearrange("p (h d) -> p h d", h=HL)
                    nc.vector.tensor_add(ob, ib, bb_)
                cp.is_dve = True
                yield cp

            def proj_group(mt_abs, ks=None, partial_out=None, partial_in=None):
                """Output projection for query block mt_abs (128 rows).

                ks selects attnT feature tiles to contract (default all).
                With partial_out, the result (+bias) is parked in SBUF instead
                of DMA'd; with partial_in, the parked partial is added on the
                way out.
                """
                ks = list(range(PT)) if ks is None else list(ks)
                ybox = {}
                for on in range(2):
                    box = {}
                    for k in ks:
                        def mm(k=k, on=on, box=box, mt_abs=mt_abs, ks=ks):
                            if "ps" not in box:
                                box["ps"] = psp.tile([128, 512], f32, tag="kp", bufs=2,
                                                     name=f"pj{mt_abs}_{on}")
                            nc.tensor.matmul(
                                box["ps"][:],
                                lhsT=attnT[:, k * NQ + mt_abs * 128 : k * NQ + (mt_abs + 1) * 128],
                                rhs=wpv[:, k, on * 512 : (on + 1) * 512],
                                start=(k == ks[0]), stop=(k == ks[-1]),
                            )
                        yield mm

                    def cp(box=box, on=on, mt_abs=mt_abs, ybox=ybox):
                        if on == 0:
                            ybox["yt"] = pb.tile([128, 1024], f32, tag="y", bufs=2,
                                                 name=f"y{mt_abs}")
                        yt = ybox["yt"]
                        nc.vector.tensor_add(
                            yt[:, on * 512 : (on + 1) * 512],
                            box["ps"][:], bpb[:, on * 512 : (on + 1) * 512])
                        if on == 1:
                            # both halves ready: one DMA per query tile (sync
                            # queue issues are ~1us each, so fewer is faster)
                            nc.sync.dma_start(
                                out_d[mt_abs * 128 : (mt_abs + 1) * 128, :],
                                yt[:],
                            )
                    cp.is_dve = True
                    yield cp

            seq = [0]
            pending = []   # (release, deadline, seq, op)

            def add_group(deadline, gen, release=0):
                for op in gen:
                    pending.append((release, deadline, seq[0], op))
                    seq[0] += 1

            # Q: pair ft's qt chunk needed by cycle qt*CYC + ft*8.
            for ft in range(NP):
                for qt in range(FQ):
                    add_group(qt * CYC + ft * 8 - 1, q_group(ft, qt))
            # K: pair ft chunk cg needed by cycle ft*8 + cg*2 (first qt pass).
            for ft in range(NP):
                for cg in range(NK // 512):
                    add_group(ft * 8 + cg * 2 - 1, k_group(ft, cg))
            # V: tile tt consumed from cycle tt//2 + LAG.
            for tt in range(TT):
                add_group(tt // 2 + LAG - 1, v_group(tt))

            def sort_pending():
                pending.sort(key=lambda t: (t[0], t[2]))
                pending.reverse()  # pop() yields lowest release first

            sort_pending()

            def emit_fillers(c, n_base=6):
                while pending and pending[-1][0] <= c:
                    _, dl, sq, op = pending.pop()
                    fillers.append((dl, sq, op))
                fillers.sort(key=lambda t: (t[0], t[1]))
                fillers.reverse()
                # Deadline-due ops are emitted UNCONDITIONALLY: a consumer
                # emitted before its producer copy would get no RAW dependency
                # at all and race it on hardware.
                n = 0
                while fillers and (fillers[-1][0] <= c or n < n_base):
                    _, _, op = fillers.pop()
                    op()
                    n += 1

            # ---------------- main loop ----------------
            pv_q = []          # pending (p, qt, kg, ptA, ptB)
            pots = {}          # head -> ot psum tile

            def emit_pv():
                p2, qt2, kg2, ptJ0, ptJ1 = pv_q.pop(0)
                for hb in range(2):
                    h = 2 * p2 + hb
                    if kg2 == 0:
                        pots[h] = psp.tile([VW, 512], f32, tag="ot", bufs=2, name=f"ot{h}_{qt2}")
                    po = pots[h]
                    for j, ptJ in ((0, ptJ0), (1, ptJ1)):
                        kt = kg2 * 2 + j
                        nc.tensor.matmul(
                            po[:],
                            lhsT=vpv[:, kt * HL + h, :],
                            rhs=ptJ[:, hb * 512 : hb * 512 + 512],
                            start=(kt == 0), stop=(kt == TT - 1),
                        )
                if kg2 == TT // 2 - 1:
                    for h in (2 * p2, 2 * p2 + 1):
                        po = pots.pop(h)
                        ft, bp = h // 2, (h % 2) * 64
                        # Spill po to SBUF immediately so the PSUM slot frees
                        # for the next pair's PV accumulation.
                        pos_t = pb.tile([64, 512], bf16, tag="pos", bufs=3)
                        nc.vector.tensor_copy(pos_t[:], po[0:DH, :])
                        # softmax denominator: 1/rowsum (ones-column of po).
                        # reciprocal_approx_fast misreads PSUM sources -- stage
                        # the row to SBUF first.
                        rc = pb.tile([1, 512], f32, tag="rc", bufs=2)
                        nc.vector.tensor_copy(rc[0:1, :], po[DH : DH + 1, :])
                        ri = pb.tile([1, 512], f32, tag="ri", bufs=2)
                        nc.vector.reciprocal_approx_fast(ri[0:1, :], rc[0:1, :])
                        bb_t = pb.tile([64, 512], f32, tag="bb", bufs=2)
                        nc.gpsimd.partition_broadcast(bb_t[:], ri[0:1, :])
                        # V carries its bias; with softmax rows summing to 1
                        # the normalize is a plain multiply.
                        nc.vector.tensor_mul(
                            attnT[bp : bp + 64,
                                  ft * NQ + qt2 * 512 : ft * NQ + qt2 * 512 + 512],
                            pos_t[:],
                            bb_t[:],
                        )

            c = 0
            for qt in range(FQ):
                for p in range(NP):
                    for kg in range(TT // 2):
                        emit_fillers(c, 6)
                        # scores packed per key-tile: one [128, 1024] psum per
                        # kt holding [head A | head B]. Both heads' matmuls
                        # share identical deps (same psum WAR, same exp), so
                        # the scheduler keeps them adjacent and the 64x128 row
                        # tiles (A at PE rows 0-63, B at 64-127) overlap.
                        pts = []
                        for j in range(2):
                            kt = kg * 2 + j
                            psJ = psp.tile([128, 1024], f32, tag="sc", bufs=2, name=f"sc{c}_{j}")
                            for hb, bp in ((0, 0), (1, 64)):
                                nc.tensor.matmul(
                                    psJ[:, hb * 512 : hb * 512 + 512],
                                    lhsT=KTs[bp : bp + 64,
                                             p * NK + kt * 128 : p * NK + (kt + 1) * 128],
                                    rhs=QT[bp : bp + 64,
                                           p * NQ + qt * 512 : p * NQ + qt * 512 + 512],
                                    start=True, stop=True,
                                )
                            ptJ = pb.tile([128, 1024], bf16, tag="pt", bufs=2 * LAG + 2, name=f"pt{c}_{j}")
                            nc.scalar.activation(ptJ[:], psJ[:], AF.Exp, scale=0.125)
                            pts.append(ptJ)
                        pv_q.append((p, qt, kg, pts[0], pts[1]))
                        if len(pv_q) > LAG:
                            emit_pv()
                        c += 1
                # qt's projection becomes legal once all its normalizes are
                # emitted; schedule it as filler work across the next qt pass.
                if qt < FQ - 1:
                    # staggered releases: the normalize chain the projection
                    # reads (DVE copy -> recip -> gpsimd broadcast -> DVE mul)
                    # has multi-cycle latency after emission; emitting proj
                    # matmuls too early head-of-line blocks the in-order PE
                    # queue on the attnT RAW dependency.
                    for mtl in range(4):
                        add_group(qt * CYC + CYC + 10 + mtl * 5,
                                  proj_group(qt * 4 + mtl),
                                  release=qt * CYC + CYC + LAG + 3 + mtl * 4)
                    sort_pending()
            # drain PV pipeline
            while pv_q:
                emit_pv()
            # keep the PE busy through the last normalize chains' drain
            # (~6us DVE+GPSIMD serial latency): an idle gap >3.4us would
            # re-throttle the HAM clock gate and the tail projections would
            # run at half clock. Fresh pool tile -- writing through the old
            # jk handle would alias a recycled psum slot.
            jk2 = psp.tile([128, 512], f32, tag="kp", bufs=2, name="junk2")
            for _ in range(80):
                nc.tensor.matmul(jk2[:, 0:128], lhsT=wup[:], rhs=wup[:], start=True, stop=True)
            # tail: remaining fillers, then the last qt's projection
            while pending:
                _, dl, sq, op = pending.pop()
                fillers.append((dl, sq, op))
            fillers.sort(key=lambda t: (t[0], t[1]))
            fillers.reverse()
            while fillers:
                _, _, op = fillers.pop()
                op()
            for mt_abs in range(4 * (FQ - 1), 4 * FQ):
                for op in proj_group(mt_abs):
                    op()

    nc.finalize()
    return nc


def _get_nc():
    if "nc" not in _CACHED:
        _CACHED["nc"] = _build()
    return _CACHED["nc"]


def kernel(x, key_padding_mask, Wqkv, bqkv, Wproj, bproj):
    x = np.asarray(x, dtype=np.float32)
    Wqkv = np.asarray(Wqkv, dtype=np.float32)
    bqkv = np.asarray(bqkv, dtype=np.float32)
    Wproj = np.asarray(Wproj, dtype=np.float32)
    bproj = np.asarray(bproj, dtype=np.float32)

    bpb = np.ascontiguousarray(
        np.broadcast_to((bproj * 0.5).reshape(1, C), (128, C))
    ).astype(np.float32)

    xTb = [np.ascontiguousarray(x[b].T).astype(ml_dtypes.bfloat16) for b in range(B)]
    halves = []
    for hh in range(2):
        sl = slice(hh * CL, (hh + 1) * CL)
        wq, wk, wvs = Wqkv[:, sl], Wqkv[:, C + hh * CL : C + (hh + 1) * CL], \
            Wqkv[:, 2 * C + hh * CL : 2 * C + (hh + 1) * CL]
        wqkv_h = np.ascontiguousarray(
            np.concatenate([wq, wk, wvs], axis=1)
        ).astype(ml_dtypes.bfloat16)
        bq = bqkv[hh * CL : (hh + 1) * CL].reshape(NP, 128)
        bk = bqkv[C + hh * CL : C + (hh + 1) * CL].reshape(NP, 128)
        bv = bqkv[2 * C + hh * CL : 2 * C + (hh + 1) * CL]
        bqkvT_h = np.ascontiguousarray(
            np.concatenate([bq, bk, bv.reshape(NP, 128)], axis=0).T
        ).astype(np.float32)
        bvb_h = np.ascontiguousarray(
            np.broadcast_to(bv.reshape(1, CL), (128, CL))
        ).astype(np.float32)
        wproj_h = np.ascontiguousarray(Wproj[hh * CL : (hh + 1) * CL, :]).astype(
            ml_dtypes.bfloat16
        )
        halves.append((wqkv_h, bqkvT_h, bvb_h, wproj_h))

    in_maps = []
    for cc in range(8):
        b, hh = cc // 2, cc % 2
        wqkv_h, bqkvT_h, bvb_h, wproj_h = halves[hh]
        in_maps.append(
            {
                "xT": xTb[b],
                "wqkv": wqkv_h,
                "bqkvT": bqkvT_h,
                "bvb": bvb_h,
                "bpb": bpb,
                "wproj": wproj_h,
            }
        )

    _CACHED["in_maps"] = in_maps
    nc = _get_nc()
    res = run_bass_kernel_spmd(nc, in_maps, core_ids=list(range(8)), trace=False)
    _CACHED["res"] = res

    out = np.empty((B, N, C), dtype=np.float32)
    for b in range(B):
        out[b] = res.results[2 * b]["out"] + res.results[2 * b + 1]["out"]
    return out
